# revision 1
# baseline (speedup 1.0000x reference)
"""T5-style attention layer (B=4, S=2048, D=1024, H=16, DK=64) on 8 trn2 cores.

Sharding: batch (4) x head-group (2 groups of 8 heads). Core c -> batch c//2,
head-group c%2. Each core computes its batch's attention output restricted to
its 8 heads, projected through its Wo row-slice -> partial [S, D] output.
Host sums the two head-group partials per batch (the "all-reduce").

On-device math (per core), matmuls in bf16 (fp32 PSUM accumulation):
  phase 1: Q^T, K^T (as [hd, s]) and V (as [s, hd]) projections from x^T.
  phase 2: per (q-chunk 512, head): S^T tiles [128 k, 512 q] = K^T.T @ Q^T in
    PSUM; T5 relative-position bias added via identity-matmul of precomputed
    Toeplitz band patterns (6 distinct k0-q0 alignments) for near-diagonal
    tiles, or folded into the exp's per-partition bias operand for tiles that
    are entirely in the bucket-saturated far region (bias there is a per-head
    constant). exp on ScalarE (no row-max needed: scores ~ N(0,1)).
    AV: O^T[dk,q] accum = [V|1].T @ exp(S^T) -> row 64 = softmax denominator.
    Normalize columns by 1/denominator (DVE + small DMA broadcast).
  phase 3 (interleaved per q-chunk): out = O_norm @ Wo accumulated over heads.
"""

import math

import ml_dtypes
import numpy as np

import concourse.bass as bass
import concourse.mybir as mybir
import concourse.tile as tile
from concourse import bacc
from concourse.bass_utils import run_bass_kernel_spmd
from concourse.masks import make_identity

import os

F32 = mybir.dt.float32
F32R = mybir.dt.float32r
BF16 = mybir.dt.bfloat16
MMDT = BF16 if os.environ.get("KERNEL_BF16", "1") == "1" else F32R
MMNP = ml_dtypes.bfloat16 if os.environ.get("KERNEL_BF16", "1") == "1" else np.float32
AF = mybir.ActivationFunctionType

B, S, D, H, DK = 4, 2048, 1024, 16, 64
HG = 8  # heads per core
HDG = HG * DK  # 512
QC = 512  # q chunk width
NKT = S // 128  # 16 k tiles
NQC = S // QC  # 4 q chunks
DBASES = [-128, 0, 128, 256, 384, 512]  # near-band k0-q0 alignments

_NC_CACHE = {}


def _build_nc():
    nc = bacc.Bacc(None, target_bir_lowering=False, debug=False)
    xT = nc.dram_tensor("xT", [D, S], MMDT, kind="ExternalInput")
    wq = nc.dram_tensor("wq", [D, HDG], MMDT, kind="ExternalInput")
    wk = nc.dram_tensor("wk", [D, HDG], MMDT, kind="ExternalInput")
    wv = nc.dram_tensor("wv", [D, HDG], MMDT, kind="ExternalInput")
    wo = nc.dram_tensor("wo", [HDG, D], MMDT, kind="ExternalInput")
    pat = nc.dram_tensor("pat", [HG, len(DBASES) + 2, 128, QC], BF16, kind="ExternalInput")
    cst = nc.dram_tensor("cst", [128, 2 * HG], F32, kind="ExternalInput")
    onesd = nc.dram_tensor("ones", [128, NKT * HG], MMDT, kind="ExternalInput")
    outd = nc.dram_tensor("out", [S, D], F32, kind="ExternalOutput")

    with tile.TileContext(nc) as tc:
        with tc.tile_pool(name="persist", bufs=1) as persist:
            qt = persist.tile([128, 4, S], MMDT, tag="qt")
            kt = persist.tile([128, 4, S], MMDT, tag="kt")
            vt = persist.tile([128, NKT, HG, DK + 1], MMDT, tag="vt")
            ident = persist.tile([128, 128], BF16, tag="ident")
            csts = persist.tile([128, 2 * HG], F32, tag="csts")
            make_identity(nc, ident)
            nc.sync.dma_start(out=csts, in_=cst[:, :])
            nc.sync.dma_start(
                out=vt[:, :, :, DK : DK + 1],
                in_=onesd.rearrange("p (a b c) -> p a b c", a=NKT, b=HG),
            )

            # ---- phase 1: projections ----
            with tc.tile_pool(name="ph1w", bufs=1) as wpool, tc.tile_pool(
                name="ph1x", bufs=2
            ) as xpool, tc.tile_pool(name="ph1ps", bufs=4, space="PSUM") as ps1:
                wqs = wpool.tile([128, 8, HDG], MMDT, tag="wqs")
                wks = wpool.tile([128, 8, HDG], MMDT, tag="wks")
                wvs = wpool.tile([128, 8, HDG], MMDT, tag="wvs")
                nc.sync.dma_start(out=wqs, in_=wq.rearrange("(dc p) n -> p dc n", p=128))
                nc.sync.dma_start(out=wks, in_=wk.rearrange("(dc p) n -> p dc n", p=128))
                nc.sync.dma_start(out=wvs, in_=wv.rearrange("(dc p) n -> p dc n", p=128))
                for sc in range(4):
                    xq = xpool.tile([128, 8, QC], MMDT, tag="xq")
                    nc.sync.dma_start(
                        out=xq,
                        in_=xT[:, sc * QC : (sc + 1) * QC].rearrange(
                            "(dc p) s -> p dc s", p=128
                        ),
                    )
                    for m in range(4):
                        for wsrc, dst in ((wqs, qt), (wks, kt)):
                            p_ps = ps1.tile([128, QC], F32, tag="ps")
                            for dc in range(8):
                                nc.tensor.matmul(
                                    p_ps,
                                    wsrc[:, dc, m * 128 : (m + 1) * 128],
                                    xq[:, dc, :],
                                    start=(dc == 0),
                                    stop=(dc == 7),
                                )
                            nc.vector.tensor_copy(
                                dst[:, m, sc * QC : (sc + 1) * QC], p_ps
                            )
                    for st in range(4):
                        v_ps = ps1.tile([128, HDG], F32, tag="ps")
                        for dc in range(8):
                            nc.tensor.matmul(
                                v_ps,
                                xq[:, dc, st * 128 : (st + 1) * 128],
                                wvs[:, dc, :],
                                start=(dc == 0),
                                stop=(dc == 7),
                            )
                        nc.vector.tensor_copy(
                            vt[:, sc * 4 + st, :, 0:DK],
                            v_ps.rearrange("p (h d) -> p h d", h=HG),
                        )

            # ---- phase 2 (attention) + phase 3 (output projection) ----
            # Head PAIRS (2m, 2m+1): the two heads' K^T/Q^T slices live in
            # partition rows 0-63 / 64-127 of the same hd-tile, so their score
            # matmuls target disjoint PE row-groups and run concurrently when
            # emitted back-to-back. One [128, 4, 512] PSUM tile holds scores
            # for (head, q-half); one ACTIVATE(Exp) per head covers 1024
            # elements. AV runs one k-tile behind the scores so the PE does
            # not wait on ScalarE. Normalization copies O^T off PSUM first so
            # the slow reciprocal stays off the critical path.
            with tc.tile_pool(name="ph2", bufs=1) as p2:
                wos = p2.tile([128, 4, D], MMDT, tag="wos")
                ot = p2.tile([128, 4, S], MMDT, tag="ot")
                nc.sync.dma_start(out=wos, in_=wo.rearrange("(m p) n -> p m n", p=128))
                with tc.tile_pool(name="patp", bufs=2) as patp, tc.tile_pool(
                    name="attnp", bufs=4
                ) as attnp, tc.tile_pool(name="rp", bufs=4) as rp, tc.tile_pool(
                    name="ps_s", bufs=1, space="PSUM"
                ) as ps_s, tc.tile_pool(name="ps_o", bufs=4, space="PSUM") as ps_o:
                    for m in range(4):
                        path = patp.tile(
                            [128, 2, len(DBASES) + 2, QC], BF16, tag="pth"
                        )
                        nc.sync.dma_start(
                            out=path,
                            in_=pat[2 * m : 2 * m + 2].rearrange(
                                "h j p c -> p h j c"
                            ),
                        )
                        for qcp in range(2):
                            o_pss = [
                                ps_o.tile([DK + 1, QC], F32, tag="ops", name=f"o{i}")
                                for i in range(4)
                            ]
                            pending = None
                            for kti in range(NKT):
                                d0 = kti * 128 - qcp * 1024
                                uniform = d0 <= -256 or d0 >= 1152
                                s_ps = ps_s.tile([128, 4, QC], F32, tag="sps")
                                if not uniform:
                                    for hh in range(2):
                                        for qc2 in range(2):
                                            db = d0 - 512 * qc2
                                            if db in DBASES:
                                                j = DBASES.index(db)
                                            elif db <= -256:
                                                j = 6
                                            else:
                                                j = 7
                                            nc.tensor.matmul(
                                                s_ps[:, hh * 2 + qc2, :],
                                                ident,
                                                path[:, hh, j, :],
                                                start=True,
                                                stop=False,
                                            )
                                for qc2 in range(2):
                                    for hh in range(2):
                                        qc = qcp * 2 + qc2
                                        nc.tensor.matmul(
                                            s_ps[:, hh * 2 + qc2, :],
                                            kt[
                                                hh * 64 : (hh + 1) * 64,
                                                m,
                                                kti * 128 : (kti + 1) * 128,
                                            ],
                                            qt[
                                                hh * 64 : (hh + 1) * 64,
                                                m,
                                                qc * QC : (qc + 1) * QC,
                                            ],
                                            start=uniform,
                                            stop=True,
                                        )
                                if pending is not None:
                                    pat_, pkti = pending
                                    for i in range(4):
                                        nc.tensor.matmul(
                                            o_pss[i],
                                            vt[:, pkti, 2 * m + i // 2, :],
                                            pat_[:, i, :],
                                            start=(pkti == 0),
                                            stop=False,
                                        )
                                at = attnp.tile([128, 4, QC], MMDT, tag="at")
                                for hh in range(2):
                                    h = 2 * m + hh
                                    if uniform:
                                        col = 2 * h + (0 if d0 <= -256 else 1)
                                        nc.scalar.activation(
                                            at[:, hh * 2 : hh * 2 + 2, :],
                                            s_ps[:, hh * 2 : hh * 2 + 2, :],
                                            AF.Exp,
                                            bias=csts[:, col : col + 1],
                                        )
                                    else:
                                        nc.scalar.activation(
                                            at[:, hh * 2 : hh * 2 + 2, :],
                                            s_ps[:, hh * 2 : hh * 2 + 2, :],
                                            AF.Exp,
                                        )
                                pending = (at, kti)
                            pat_, pkti = pending
                            for i in range(4):
                                nc.tensor.matmul(
                                    o_pss[i],
                                    vt[:, pkti, 2 * m + i // 2, :],
                                    pat_[:, i, :],
                                    start=False,
                                    stop=True,
                                )
                            for i in range(4):
                                hh, qc2 = i // 2, i % 2
                                qc = qcp * 2 + qc2
                                oc = rp.tile([DK + 1, QC], F32, tag="oc", name=f"oc{i}")
                                nc.vector.tensor_copy(oc, o_pss[i])
                                r1 = rp.tile([1, QC], F32, tag="r1")
                                nc.vector.reciprocal(r1, oc[DK : DK + 1, :])
                                rb = rp.tile([64, QC], F32, tag="rb")
                                r1ap = r1[0:1, :]
                                nc.sync.dma_start(
                                    out=rb,
                                    in_=bass.AP(
                                        tensor=r1ap.tensor,
                                        offset=r1ap.offset,
                                        ap=[
                                            [list(r1ap.ap[0])[0], 1],
                                            [0, 64],
                                            list(r1ap.ap[-1]),
                                        ],
                                    ),
                                )
                                nc.vector.tensor_mul(
                                    ot[
                                        hh * 64 : (hh + 1) * 64,
                                        m,
                                        qc * QC : (qc + 1) * QC,
                                    ],
                                    oc[0:DK, :],
                                    rb,
                                )
                # ---- phase 3: output projection ----
                with tc.tile_pool(name="outb", bufs=4) as outb, tc.tile_pool(
                    name="ps_out", bufs=4, space="PSUM"
                ) as ps_out:
                    for st_g in range(16):
                        for nck in range(2):
                            out_ps = ps_out.tile([128, 512], F32, tag="outps")
                            for m2 in range(4):
                                nc.tensor.matmul(
                                    out_ps,
                                    ot[:, m2, st_g * 128 : (st_g + 1) * 128],
                                    wos[:, m2, nck * 512 : (nck + 1) * 512],
                                    start=(m2 == 0),
                                    stop=(m2 == 3),
                                )
                            ob = outb.tile([128, 512], F32, tag="ob")
                            nc.vector.tensor_copy(ob, out_ps)
                            nc.sync.dma_start(
                                out=outd[
                                    st_g * 128 : (st_g + 1) * 128,
                                    nck * 512 : (nck + 1) * 512,
                                ],
                                in_=ob,
                            )
    nc.compile()
    return nc


def _bias_offsets(rel_bias_table):
    """bias value per relative offset d = k - q in [-2047, 2047] -> [H, 4095].

    Mirrors reference._relative_position_bucket op-for-op in jax so that the
    bucket indices match the grading reference bit-exactly (the jax backend's
    jnp.log is an approximation, so host numpy log can flip int-cast
    boundaries).
    """
    import jax.numpy as jnp

    d = jnp.arange(-(S - 1), S)
    nb = 16
    buckets = (d > 0).astype(jnp.int32) * nb
    rp = jnp.abs(d)
    max_exact = nb // 2
    is_small = rp < max_exact
    rl = max_exact + (
        jnp.log(jnp.maximum(rp, 1).astype(jnp.float32) / max_exact)
        / math.log(128 / max_exact)
        * (nb - max_exact)
    ).astype(jnp.int32)
    rl = jnp.minimum(rl, nb - 1)
    bucket = np.asarray(buckets + jnp.where(is_small, rp, rl))  # [4095]
    return np.asarray(rel_bias_table)[bucket, :].T.astype(np.float32)  # [H, 4095]


def kernel(hidden_states, Wq, Wk, Wv, Wo, rel_bias_table, _trace=False):
    hidden_states = np.ascontiguousarray(hidden_states, dtype=np.float32)
    Wq = np.asarray(Wq, dtype=np.float32)
    Wk = np.asarray(Wk, dtype=np.float32)
    Wv = np.asarray(Wv, dtype=np.float32)
    Wo = np.asarray(Wo, dtype=np.float32)
    rel_bias_table = np.asarray(rel_bias_table, dtype=np.float32)

    if "nc" not in _NC_CACHE:
        _NC_CACHE["nc"] = _build_nc()
    nc = _NC_CACHE["nc"]

    bias_off = _bias_offsets(rel_bias_table)  # [H, 4095]
    # patterns[g][h, j, p, c] = bias(d = DBASES[j] + p - c) for head g*8+h
    pidx = (
        np.array(DBASES)[None, :, None, None]
        + np.arange(128)[None, None, :, None]
        - np.arange(QC)[None, None, None, :]
        + (S - 1)
    )  # [1, 6, 128, 512]
    in_maps = []
    for core in range(8):
        b, g = core // 2, core % 2
        heads = slice(g * HG, (g + 1) * HG)
        pat6 = bias_off[heads][
            np.arange(HG)[:, None, None, None], pidx
        ]  # [8, 6, 128, 512]
        pat = np.zeros((HG, 8, 128, QC), dtype=np.float32)
        pat[:, :6] = pat6
        for h in range(HG):
            pat[h, 6] = rel_bias_table[15, g * HG + h]
            pat[h, 7] = rel_bias_table[31, g * HG + h]
        cst = np.zeros((128, 2 * HG), dtype=np.float32)
        for h in range(HG):
            cst[:, 2 * h] = rel_bias_table[15, g * HG + h]  # far-left bucket
            cst[:, 2 * h + 1] = rel_bias_table[31, g * HG + h]  # far-right bucket
        in_maps.append(
            {
                "xT": np.ascontiguousarray(hidden_states[b].T).astype(MMNP),
                "wq": np.ascontiguousarray(Wq[:, g * HDG : (g + 1) * HDG]).astype(MMNP),
                "wk": np.ascontiguousarray(Wk[:, g * HDG : (g + 1) * HDG]).astype(MMNP),
                "wv": np.ascontiguousarray(Wv[:, g * HDG : (g + 1) * HDG]).astype(MMNP),
                "wo": np.ascontiguousarray(Wo[g * HDG : (g + 1) * HDG, :]).astype(MMNP),
                "pat": np.ascontiguousarray(pat.astype(ml_dtypes.bfloat16)),
                "cst": cst,
                "ones": np.ones((128, NKT * HG), dtype=MMNP),
            }
        )

    res = run_bass_kernel_spmd(nc, in_maps, core_ids=list(range(8)), trace=_trace)
    global LAST_RESULTS
    LAST_RESULTS = res
    out = np.empty((B, S, D), dtype=np.float32)
    for b in range(B):
        out[b] = res.results[2 * b]["out"] + res.results[2 * b + 1]["out"]
    return out


LAST_RESULTS = None



# revision 10
# speedup vs baseline: 1.2297x; 1.2297x over previous
"""T5-style attention layer (B=4, S=2048, D=1024, H=16, DK=64) on 8 trn2 cores.

Sharding: batch (4) x head-group (2 groups of 8 heads). Core c -> batch c//2,
head-group c%2. Each core computes its batch's attention output restricted to
its 8 heads, projected through its Wo row-slice -> partial [S, D] output.
Host sums the two head-group partials per batch (the "all-reduce").

On-device math (per core), matmuls in bf16 (fp32 PSUM accumulation):
  phase 1: Q^T, K^T (as [hd, s]) and V (as [s, hd]) projections from x^T.
    V is kept in three copies: plain, and pre-scaled by exp(bias_left[h]) /
    exp(bias_right[h]) so that score tiles that sit entirely in a
    bucket-saturated region need no bias injection at all (the per-head
    constant multiplies through exp(s+c) = e^c exp(s) into the AV stage,
    including the appended ones-row that forms the softmax denominator).
  phase 2: per (head-pair m, 512-wide q chunk, 128-wide k tile): scores^T
    [128 k, 2 heads, 512 q] in double-buffered PSUM; near-diagonal tiles get
    the exact T5 relative-position bias added via identity-matmul of
    precomputed Toeplitz band patterns (6 distinct alignments). One
    unbiased ACTIVATE(Exp) per k tile covers both heads. AV (O^T[65, q] +=
    [V|1].T @ exp(S^T)) runs one k-tile behind the scores. The denominator
    row is transposed to [128, 4] via a strided DMA so the reciprocal runs
    on all 128 DVE lanes, then broadcast back via small DMAs; one DVE
    multiply normalizes straight out of PSUM into the bf16 O^T buffer.
  phase 3: out = O_norm^T.T @ Wo per 128-row s-chunk, interleaved into the
    tail of phase 2 (m=3) where the PE has slack. Q/K projections for head
    pair m+1 are likewise interleaved into phase 2 of head pair m.
"""

import math

import ml_dtypes
import numpy as np

import concourse.bass as bass
import concourse.mybir as mybir
import concourse.tile as tile
from concourse import bacc
from concourse.bass_utils import run_bass_kernel_spmd
from concourse.masks import make_identity

F32 = mybir.dt.float32
BF16 = mybir.dt.bfloat16
MMDT = BF16
MMNP = ml_dtypes.bfloat16
AF = mybir.ActivationFunctionType

B, S, D, H, DK = 4, 2048, 1024, 16, 64
HG = 8  # heads per core
HDG = HG * DK  # 512
QC = 512  # q chunk width
NKT = S // 128  # 16 k tiles
NQC = S // QC  # 4 q chunks
DBASES = [-128, 0, 128, 256, 384, 512]  # near-band k0-q0 alignments

_NC_CACHE = {}


def _tile_side(qc, kti):
    """Classify a [128 k, 512 q] score tile: banded j, or 'L'/'R' saturated."""
    d0 = kti * 128 - qc * QC
    if d0 in DBASES:
        return DBASES.index(d0)
    return "L" if d0 <= -256 else "R"


def _build_nc():
    nc = bacc.Bacc(None, target_bir_lowering=False, debug=False)
    xT = nc.dram_tensor("xT", [D, S], MMDT, kind="ExternalInput")
    wq = nc.dram_tensor("wq", [D, HDG], MMDT, kind="ExternalInput")
    wk = nc.dram_tensor("wk", [D, HDG], MMDT, kind="ExternalInput")
    wv = nc.dram_tensor("wv", [D, HDG], MMDT, kind="ExternalInput")
    wo = nc.dram_tensor("wo", [HDG, D], MMDT, kind="ExternalInput")
    pat = nc.dram_tensor("pat", [HG, len(DBASES), 128, QC], BF16, kind="ExternalInput")
    scl = nc.dram_tensor("scl", [128, 2, HG, DK + 1], BF16, kind="ExternalInput")
    onesd = nc.dram_tensor("ones", [128, NKT * HG], MMDT, kind="ExternalInput")
    outd = nc.dram_tensor("out", [S, D], BF16, kind="ExternalOutput")

    with tile.TileContext(nc) as tc:
        with tc.tile_pool(name="persist", bufs=1) as persist:
            xs = persist.tile([128, 8, S], MMDT, tag="xs")
            qt = persist.tile([128, 4, S], MMDT, tag="qt")
            kt = persist.tile([128, 4, S], MMDT, tag="kt")
            vt = persist.tile([128, NKT, HG, DK + 1], MMDT, tag="vt")
            vtL = persist.tile([128, NKT, HG, DK + 1], MMDT, tag="vtL")
            vtR = persist.tile([128, NKT, HG, DK + 1], MMDT, tag="vtR")
            ot = persist.tile([128, 4, S], MMDT, tag="ot")
            wqs = persist.tile([128, 8, HDG], MMDT, tag="wqs")
            wks = persist.tile([128, 8, HDG], MMDT, tag="wks")
            wvs = persist.tile([128, 8, HDG], MMDT, tag="wvs")
            wos = persist.tile([128, 4, D], MMDT, tag="wos")
            scls = persist.tile([128, 2, HG, DK + 1], BF16, tag="scls")
            ident = persist.tile([128, 128], BF16, tag="ident")
            make_identity(nc, ident)

            # bulk input loads
            for sc in range(4):
                nc.sync.dma_start(
                    out=xs[:, :, sc * QC : (sc + 1) * QC],
                    in_=xT[:, sc * QC : (sc + 1) * QC].rearrange(
                        "(dc p) s -> p dc s", p=128
                    ),
                )
            nc.sync.dma_start(out=wqs, in_=wq.rearrange("(dc p) n -> p dc n", p=128))
            nc.sync.dma_start(out=wks, in_=wk.rearrange("(dc p) n -> p dc n", p=128))
            nc.sync.dma_start(out=wvs, in_=wv.rearrange("(dc p) n -> p dc n", p=128))
            nc.sync.dma_start(out=wos, in_=wo.rearrange("(m p) n -> p m n", p=128))
            nc.sync.dma_start(out=scls, in_=scl[:, :, :, :])
            nc.sync.dma_start(
                out=vt[:, :, :, DK : DK + 1],
                in_=onesd.rearrange("p (a b c) -> p a b c", a=NKT, b=HG),
            )

            # ---- phase 1: V projection (all s tiles) + Q/K for m=0 ----
            def emit_v(ph1ps, st_abs):
                sc, st = st_abs // 4, st_abs % 4
                v_ps = ph1ps.tile([128, HDG], F32, tag="vps")
                for dc in range(8):
                    nc.tensor.matmul(
                        v_ps,
                        xs[:, dc, st_abs * 128 : (st_abs + 1) * 128],
                        wvs[:, dc, :],
                        start=(dc == 0),
                        stop=(dc == 7),
                    )
                nc.vector.tensor_copy(
                    vt[:, st_abs, :, 0:DK],
                    v_ps.rearrange("p (h d) -> p h d", h=HG),
                )
                # scaled copies (after ones column is present)
                nc.vector.tensor_mul(vtL[:, st_abs], vt[:, st_abs], scls[:, 0])
                nc.vector.tensor_mul(vtR[:, st_abs], vt[:, st_abs], scls[:, 1])

            def emit_qk(ph1ps, m, sc, wsrc, dst):
                p_ps = ph1ps.tile([128, QC], F32, tag="qkps")
                for dc in range(8):
                    nc.tensor.matmul(
                        p_ps,
                        wsrc[:, dc, m * 128 : (m + 1) * 128],
                        xs[:, dc, sc * QC : (sc + 1) * QC],
                        start=(dc == 0),
                        stop=(dc == 7),
                    )
                nc.vector.tensor_copy(dst[:, m, sc * QC : (sc + 1) * QC], p_ps)

            with tc.tile_pool(name="ph1ps", bufs=4, space="PSUM") as ph1ps:
                for st_abs in range(NKT):
                    emit_v(ph1ps, st_abs)
                for sc in range(4):
                    emit_qk(ph1ps, 0, sc, wqs, qt)
                    emit_qk(ph1ps, 0, sc, wks, kt)

            # ---- phase 2 (+ interleaved QK for m+1, phase 3 for m=3) ----
            with tc.tile_pool(name="patp", bufs=2) as patp, tc.tile_pool(
                name="attnp", bufs=3
            ) as attnp, tc.tile_pool(name="normp", bufs=2) as normp, tc.tile_pool(
                name="ps_s", bufs=2, space="PSUM"
            ) as ps_s, tc.tile_pool(name="obp", bufs=2) as obp:
                for m in range(4):
                    path = patp.tile([128, 2, len(DBASES), QC], BF16, tag="pth")
                    nc.sync.dma_start(
                        out=path,
                        in_=pat[2 * m : 2 * m + 2].rearrange("h j p c -> p h j c"),
                    )
                    # deferred work units to interleave into this m's phase 2
                    if m < 3:
                        defer = [
                            (emit_qk, (m + 1, sc, wsrc, dst))
                            for sc in range(4)
                            for wsrc, dst in ((wqs, qt), (wks, kt))
                        ]
                    else:
                        defer = []  # phase-3 units appended per qc below

                    o_bufs = 3 if m < 3 else 2
                    with tc.tile_pool(
                        name=f"ps_o{m}", bufs=o_bufs, space="PSUM"
                    ) as ps_o, (
                        tc.tile_pool(name=f"ps_i{m}", bufs=1, space="PSUM")
                        if m < 3
                        else _null_ctx()
                    ) as ps_i, (
                        tc.tile_pool(name="ps_out", bufs=2, space="PSUM")
                        if m == 3
                        else _null_ctx()
                    ) as ps_out:
                        def emit_out_chunk(st_g, nck):
                            out_ps = ps_out.tile([128, 512], F32, tag="outps")
                            for m2 in range(4):
                                nc.tensor.matmul(
                                    out_ps,
                                    ot[:, m2, st_g * 128 : (st_g + 1) * 128],
                                    wos[:, m2, nck * 512 : (nck + 1) * 512],
                                    start=(m2 == 0),
                                    stop=(m2 == 3),
                                )
                            ob = obp.tile([128, 512], BF16, tag="ob")
                            nc.vector.tensor_copy(ob, out_ps)
                            nc.sync.dma_start(
                                out=outd[
                                    st_g * 128 : (st_g + 1) * 128,
                                    nck * 512 : (nck + 1) * 512,
                                ],
                                in_=ob,
                            )

                        for qc in range(NQC):
                            o_pss = [
                                ps_o.tile([DK + 1, QC], F32, tag="ops", name=f"o{hh}")
                                for hh in range(2)
                            ]
                            pending = None

                            def do_av(pend):
                                pat_, pkti = pend
                                side = _tile_side(qc, pkti)
                                vsrc = vt if isinstance(side, int) else (
                                    vtL if side == "L" else vtR
                                )
                                for hh in range(2):
                                    nc.tensor.matmul(
                                        o_pss[hh],
                                        vsrc[:, pkti, 2 * m + hh, :],
                                        pat_[:, hh, :],
                                        start=(pkti == 0),
                                        stop=(pkti == NKT - 1),
                                    )

                            for kti in range(NKT):
                                side = _tile_side(qc, kti)
                                s_ps = ps_s.tile([128, 2, QC], F32, tag="sps")
                                if isinstance(side, int):
                                    for hh in range(2):
                                        nc.tensor.matmul(
                                            s_ps[:, hh, :],
                                            ident,
                                            path[:, hh, side, :],
                                            start=True,
                                            stop=False,
                                        )
                                for hh in range(2):
                                    nc.tensor.matmul(
                                        s_ps[:, hh, :],
                                        kt[
                                            hh * 64 : (hh + 1) * 64,
                                            m,
                                            kti * 128 : (kti + 1) * 128,
                                        ],
                                        qt[
                                            hh * 64 : (hh + 1) * 64,
                                            m,
                                            qc * QC : (qc + 1) * QC,
                                        ],
                                        start=not isinstance(side, int),
                                        stop=True,
                                    )
                                if pending is not None:
                                    do_av(pending)
                                # interleaved deferred unit (QK-next / phase-3)
                                cadence = 8 if m < 3 else 2
                                if defer and kti % cadence == 1:
                                    fn, args = defer.pop(0)
                                    if fn is emit_qk:
                                        fn(ps_i, *args)
                                    else:
                                        fn(*args)
                                at = attnp.tile([128, 2, QC], MMDT, tag="at")
                                nc.scalar.activation(at, s_ps, AF.Exp)
                                pending = (at, kti)
                            do_av(pending)

                            # ---- normalization for this (m, qc) ----
                            for hh in range(2):
                                o_ps = o_pss[hh]
                                tden = normp.tile([128, 4], F32, tag="tden")
                                rsb = normp.tile([128, 4], F32, tag="rsb")
                                rrow = normp.tile([1, QC], F32, tag="rrow")
                                rb = normp.tile([64, QC], F32, tag="rb")
                                drow = normp.tile([1, QC], F32, tag="drow")
                                nc.vector.tensor_copy(drow, o_ps[DK : DK + 1, :])
                                dap = drow[0:1, :]
                                # transpose den row -> [128, 4] (partition-major)
                                nc.sync.dma_start(
                                    out=tden,
                                    in_=bass.AP(
                                        tensor=dap.tensor,
                                        offset=dap.offset,
                                        ap=[
                                            [list(dap.ap[0])[0], 1],
                                            [4, 128],
                                            [1, 4],
                                        ],
                                    ),
                                )
                                nc.vector.reciprocal(rsb, tden)
                                # de-transpose back to a [1, 512] row
                                rsap = rsb[:, :]
                                rrap = rrow[0:1, :]
                                nc.sync.dma_start(
                                    out=bass.AP(
                                        tensor=rrap.tensor,
                                        offset=rrap.offset,
                                        ap=[
                                            [list(rrap.ap[0])[0], 1],
                                            [4, 128],
                                            [1, 4],
                                        ],
                                    ),
                                    in_=rsap,
                                )
                                # broadcast along 64 partitions
                                nc.sync.dma_start(
                                    out=rb,
                                    in_=bass.AP(
                                        tensor=rrap.tensor,
                                        offset=rrap.offset,
                                        ap=[
                                            [list(rrap.ap[0])[0], 1],
                                            [0, 64],
                                            list(rrap.ap[-1]),
                                        ],
                                    ),
                                )
                                nc.vector.tensor_mul(
                                    ot[
                                        hh * 64 : (hh + 1) * 64,
                                        m,
                                        qc * QC : (qc + 1) * QC,
                                    ],
                                    o_ps[0:DK, :],
                                    rb,
                                )
                            if m == 3:
                                # phase-3 for the s-range finished at qc-1
                                defer.extend(
                                    (emit_out_chunk, (st_g, nck))
                                    for st_g in range(4 * qc, 4 * qc + 4)
                                    for nck in range(2)
                                )
                        # tail: remaining deferred phase-3 chunks (last qc)
                        for fn, args in defer:
                            if fn is emit_qk:
                                fn(ps_i, *args)
                            else:
                                fn(*args)
    nc.compile()
    return nc


class _null_ctx:
    def __enter__(self):
        return None

    def __exit__(self, *a):
        return False


def _bias_offsets(rel_bias_table):
    """bias value per relative offset d = k - q in [-2047, 2047] -> [H, 4095].

    Mirrors reference._relative_position_bucket op-for-op in jax so that the
    bucket indices match the grading reference bit-exactly (the jax backend's
    jnp.log is an approximation, so host numpy log can flip int-cast
    boundaries).
    """
    import jax.numpy as jnp

    d = jnp.arange(-(S - 1), S)
    nb = 16
    buckets = (d > 0).astype(jnp.int32) * nb
    rp = jnp.abs(d)
    max_exact = nb // 2
    is_small = rp < max_exact
    rl = max_exact + (
        jnp.log(jnp.maximum(rp, 1).astype(jnp.float32) / max_exact)
        / math.log(128 / max_exact)
        * (nb - max_exact)
    ).astype(jnp.int32)
    rl = jnp.minimum(rl, nb - 1)
    bucket = np.asarray(buckets + jnp.where(is_small, rp, rl))  # [4095]
    return np.asarray(rel_bias_table)[bucket, :].T.astype(np.float32)  # [H, 4095]


def kernel(hidden_states, Wq, Wk, Wv, Wo, rel_bias_table, _trace=False):
    hidden_states = np.ascontiguousarray(hidden_states, dtype=np.float32)
    Wq = np.asarray(Wq, dtype=np.float32)
    Wk = np.asarray(Wk, dtype=np.float32)
    Wv = np.asarray(Wv, dtype=np.float32)
    Wo = np.asarray(Wo, dtype=np.float32)
    rel_bias_table = np.asarray(rel_bias_table, dtype=np.float32)

    if "nc" not in _NC_CACHE:
        _NC_CACHE["nc"] = _build_nc()
    nc = _NC_CACHE["nc"]

    bias_off = _bias_offsets(rel_bias_table)  # [H, 4095]
    # patterns[g][h, j, p, c] = bias(d = DBASES[j] + p - c) for head g*8+h
    pidx = (
        np.array(DBASES)[None, :, None, None]
        + np.arange(128)[None, None, :, None]
        - np.arange(QC)[None, None, None, :]
        + (S - 1)
    )  # [1, 6, 128, 512]
    in_maps = []
    for core in range(8):
        b, g = core // 2, core % 2
        heads = slice(g * HG, (g + 1) * HG)
        pat6 = bias_off[heads][
            np.arange(HG)[:, None, None, None], pidx
        ]  # [8, 6, 128, 512]
        # exp of saturated per-head constants, broadcast into V-shaped tiles
        scl = np.zeros((128, 2, HG, DK + 1), dtype=np.float32)
        for h in range(HG):
            scl[:, 0, h, :] = math.exp(rel_bias_table[15, g * HG + h])  # far left
            scl[:, 1, h, :] = math.exp(rel_bias_table[31, g * HG + h])  # far right
        in_maps.append(
            {
                "xT": np.ascontiguousarray(hidden_states[b].T).astype(MMNP),
                "wq": np.ascontiguousarray(Wq[:, g * HDG : (g + 1) * HDG]).astype(MMNP),
                "wk": np.ascontiguousarray(Wk[:, g * HDG : (g + 1) * HDG]).astype(MMNP),
                "wv": np.ascontiguousarray(Wv[:, g * HDG : (g + 1) * HDG]).astype(MMNP),
                "wo": np.ascontiguousarray(Wo[g * HDG : (g + 1) * HDG, :]).astype(MMNP),
                "pat": np.ascontiguousarray(pat6.astype(ml_dtypes.bfloat16)),
                "scl": scl.astype(ml_dtypes.bfloat16),
                "ones": np.ones((128, NKT * HG), dtype=MMNP),
            }
        )

    res = run_bass_kernel_spmd(nc, in_maps, core_ids=list(range(8)), trace=_trace)
    global LAST_RESULTS
    LAST_RESULTS = res
    out = np.empty((B, S, D), dtype=np.float32)
    for b in range(B):
        out[b] = res.results[2 * b]["out"].astype(np.float32) + res.results[
            2 * b + 1
        ]["out"].astype(np.float32)
    return out


LAST_RESULTS = None
